# revision 2
# baseline (speedup 1.0000x reference)
"""Trainium2 Bass kernel for nn_CCM: per-pixel complex 3x3 conv mask.

Math (per batch element b, sharded 1 batch element per NeuronCore):
  y[t,f] = sum_{n=0..8} A_n[t,f] * X[t+i(n)-2, f+j(n)-1]   (complex)
with A_n = m_n + w * m_{9+n} + conj(w) * m_{18+n}, w = -1/2 + i*sqrt(3)/2:
  Ar_n = m_n - 0.5*(m_{9+n} + m_{18+n})
  Ai_n = s * (m_{9+n} - m_{18+n}),  s = sqrt(3)/2
X = xr + i*xi, zero padded (causal in t: 2 top; symmetric in f: 1,1).

Implementation notes:
- bf16 everywhere on the DVE so tensor_tensor ops run in 2x_1p mode
  (hardware-verified: row-start alignment within a multi-dim AP does not
  matter, only inner step 1 + 16-bit dtype).
- Layout: t = 8*p + tau, partitions p in [0,125); m and acc tiles are flat
  (tau, f) rows of 257; x planes have 260-wide slot rows with f origin at
  col 1, and the three f-shifts read at col offsets {0, 1, 2} (2x_1p mode
  tolerates unaligned starts, only inner step 1 matters).
- All HBM loads go through gpsimd SWDGE casting DMAs (fp32->bf16 in
  flight), which spread descriptors across all 16 SDMA engines; the two
  HWDGE rings only reach engines 0-4 on this runtime.
- The -0.5/s basis scales run on the otherwise idle Scalar engine.
"""

import sys
import numpy as np

sys.path.insert(0, "/opt/trn_rl_repo")

B = 8
C = 27
T = 1000
F = 257
TP = 125          # partitions
TAU = 8           # t = 8*p + tau
NS = 10           # slots in x planes: tau in [-2, 8)
SROW = 260        # x plane slot row width (elements)
MW = TAU * F      # 2056: m / acc tile width (flat)
PLW = NS * SROW   # 2600: x plane width
SQ3H = float(np.sqrt(3.0) / 2.0)

_CACHE = {}


def _emit(ctx, tc, m_ap, x_ap, id_ap, y_ap):
    import concourse.mybir as mybir

    nc = tc.nc
    f32 = mybir.dt.float32
    bf16 = mybir.dt.bfloat16
    FCS = [(0, 128), (128, 128), (256, 1)]   # f chunks for transposes
    SLOT_GROUPS = [(0, 4), (4, 4), (8, 2)]   # batches of slots per psum tile

    const = ctx.enter_context(tc.tile_pool(name="const", bufs=1))
    mcpool = ctx.enter_context(tc.tile_pool(name="mc", bufs=12))
    prep = ctx.enter_context(tc.tile_pool(name="prep", bufs=2))
    prod = ctx.enter_context(tc.tile_pool(name="prod", bufs=3))
    yop = ctx.enter_context(tc.tile_pool(name="yop", bufs=2))
    psum = ctx.enter_context(tc.tile_pool(name="psum", bufs=5, space="PSUM"))
    psum2 = ctx.enter_context(tc.tile_pool(name="psum2", bufs=3, space="PSUM"))

    # ---- x and ident first: the x loads go on the SWDGE ring ahead of the
    # m-load flood (casting fp32->bf16 in flight; planes are bf16 anyway).
    ident = const.tile([128, 128], f32, tag="ident")
    nc.sync.dma_start(ident[:], id_ap)
    identb = const.tile([128, 128], bf16, tag="identb")
    nc.scalar.copy(identb[:], ident[:])

    # interleave the x chunk loads with the first taps' m loads so neither
    # path alone gates the loop start
    mtiles = {}

    def load_m(c):
        mc = mcpool.tile([TP, MW], bf16, tag="mc", name=f"mc{c}")
        nc.gpsimd.dma_start(
            mc[:], m_ap[c].rearrange("(p t) f -> p (t f)", p=TP)
        )
        mtiles[c] = mc

    xns = []
    for ci, (f0, fw) in enumerate(FCS):
        xn = const.tile([fw, (T + 2) * 2], bf16, tag=f"xn{f0}")
        nc.vector.memset(xn[:, 0:4], 0.0)
        nc.gpsimd.dma_start(
            xn[:, 4:], x_ap[f0:f0 + fw].rearrange("f t c -> f (t c)")
        )
        xns.append(xn)
        load_m((9, 18, 0)[ci])


    # ---- x planes (bf16): f origin at col 1, zero pads at cols 0, 258, 259
    planes = {}
    for nm in ("xr", "xi"):
        p = const.tile([TP, PLW], bf16, tag=nm)
        pv = p.rearrange("p (s w) -> p s w", w=SROW)
        nc.vector.memset(pv[:, :, 0:1], 0.0)
        nc.vector.memset(pv[:, :, 258:260], 0.0)
        planes[nm] = p
    for nm in ("padA", "padB"):
        const.tile([TP, PLW], bf16, tag=nm, name=nm)

    # ---- transpose x into the planes: PE (grouped into psum) + ACT copies.
    # Slot group (8,2) last: it is only read from tap i=1 onward.
    for (g0, gn) in SLOT_GROUPS:
        for ci, (f0, fw) in enumerate(FCS):
            xn3 = xns[ci].rearrange("f (t c) -> f t c", c=2)
            for q, nm in enumerate(("xr", "xi")):
                pA = planes[nm].rearrange("p (s w) -> p s w", w=SROW)
                ptg = psum.tile([TP, 512], bf16, tag="ptg")
                for u in range(gn):
                    ts = g0 + u
                    nc.tensor.transpose(
                        ptg[0:TP, 128 * u:128 * u + fw],
                        xn3[0:fw, ts:ts + TAU * (TP - 1) + 1:TAU, q],
                        identb[0:fw, 0:fw],
                    )
                src = ptg.rearrange("p (u w) -> p u w", w=128)[0:TP, 0:gn, 0:fw]
                dst = pA[:, g0:g0 + gn, 1 + f0:1 + f0 + fw]
                if g0 == 0:
                    # DVE is idle in the head; halve the ACT copy phase
                    nc.vector.tensor_copy(dst, src)
                else:
                    nc.scalar.copy(dst, src)

    # ---- remaining m tiles: SWDGE casting DMA fp32 -> bf16 into flat tiles
    # (contiguous per-partition runs keep descriptors big), used directly.
    for n in range(9):
        for c in (9 + n, 18 + n, n):
            if c not in mtiles:
                load_m(c)


    # ---- tap loop. The t1/d sums and their ACT-scaled versions (th, ds)
    # for tap n+1 are issued before tap n's products, so the DVE->ACT->DVE
    # round trip never blocks the in-order DVE queue.
    accr = const.tile([TP, MW], bf16, tag="accr")
    acci = const.tile([TP, MW], bf16, tag="acci")
    accr8 = accr.rearrange("p (r w) -> p r w", w=F)
    acci8 = acci.rearrange("p (r w) -> p r w", w=F)

    def prep_sums(n):
        m9, m18 = mtiles[9 + n], mtiles[18 + n]
        t1 = prep.tile([TP, MW], bf16, tag="t1", bufs=2, name="t1")
        nc.gpsimd.tensor_add(t1[:], m9[:], m18[:])
        th = prep.tile([TP, MW], bf16, tag="th", bufs=2, name="th")
        nc.scalar.mul(th[:], t1[:], -0.5)
        d = prep.tile([TP, MW], bf16, tag="d", bufs=2, name="d")
        nc.gpsimd.tensor_sub(d[:], m9[:], m18[:])
        ds = prep.tile([TP, MW], bf16, tag="ds", bufs=2, name="ds")
        nc.scalar.mul(ds[:], d[:], SQ3H)  # = Ai_n
        return th, ds

    nxt = prep_sums(0)
    for n in range(C // 3):
        i, j = divmod(n, 3)
        xrv = planes["xr"].rearrange("p (s w) -> p s w", w=SROW)[:, i:i + TAU, j:j + F]
        xiv = planes["xi"].rearrange("p (s w) -> p s w", w=SROW)[:, i:i + TAU, j:j + F]
        th, ds = nxt
        ar = prep.tile([TP, MW], bf16, tag="ar", bufs=2)
        nc.vector.tensor_add(ar[:], th[:], mtiles[n][:])
        if n + 1 < C // 3:
            nxt = prep_sums(n + 1)

        ar8 = ar.rearrange("p (r w) -> p r w", w=F)
        ds8 = ds.rearrange("p (r w) -> p r w", w=F)

        # The last tap runs in two tau-halves so the first half of the
        # accumulators is final early and its output transposes overlap
        # the second half's products.
        halves = ((0, 4), (4, 4)) if n == 8 else ((0, TAU),)
        for (r0, rn) in halves:
            fl = slice(r0 * F, (r0 + rn) * F)
            xrh = xrv[:, r0:r0 + rn, :]
            xih = xiv[:, r0:r0 + rn, :]
            arh = ar8[:, r0:r0 + rn, :]
            dsh = ds8[:, r0:r0 + rn, :]
            if n == 0:
                nc.vector.tensor_mul(accr8[:, r0:r0 + rn, :], arh, xrh)
            else:
                p0 = prod.tile([TP, rn * F], bf16, tag="prod", name="p0")
                nc.vector.tensor_mul(
                    p0.rearrange("p (r w) -> p r w", w=F), arh, xrh)
                nc.vector.tensor_add(accr[:, fl], accr[:, fl], p0[:])
            p2 = prod.tile([TP, rn * F], bf16, tag="prod", name="p2")
            nc.vector.tensor_mul(p2.rearrange("p (r w) -> p r w", w=F), dsh, xih)
            nc.vector.tensor_sub(accr[:, fl], accr[:, fl], p2[:])
            if n == 0:
                nc.vector.tensor_mul(acci8[:, r0:r0 + rn, :], arh, xih)
            else:
                p1 = prod.tile([TP, rn * F], bf16, tag="prod", name="p1")
                nc.vector.tensor_mul(
                    p1.rearrange("p (r w) -> p r w", w=F), arh, xih)
                nc.vector.tensor_add(acci[:, fl], acci[:, fl], p1[:])
            p3 = prod.tile([TP, rn * F], bf16, tag="prod", name="p3")
            nc.vector.tensor_mul(p3.rearrange("p (r w) -> p r w", w=F), dsh, xrh)
            nc.vector.tensor_add(acci[:, fl], acci[:, fl], p3[:])

    # ---- transpose back to [f, (t, comp)] and store; tau rows 0-3 first
    # (they are final one half-tap earlier than rows 4-7). f chunks 0 and 1
    # share one SBUF tile and a single SWDGE store (one desc-gen, spread
    # across all 16 SDMA engines).
    yo01 = yop.tile([128, 2 * T * 2], f32, tag="yo01", name="yo01")
    yo2 = yop.tile([1, T * 2], f32, tag="yo2", name="yo2")
    yviews = [
        yo01[:, 0:T * 2].rearrange("f (t c) -> f t c", c=2),
        yo01[:, T * 2:].rearrange("f (t c) -> f t c", c=2),
        yo2.rearrange("f (t c) -> f t c", c=2),
    ]
    for rg in (range(0, 4), range(4, TAU)):
        for ci, (f0, fw) in enumerate(FCS):
            yv = yviews[ci]
            for comp, acc in ((0, accr), (1, acci)):
                accv = acc.rearrange("p (r w) -> p r w", w=F)
                for r in rg:
                    pt2 = psum2.tile([128, TP], bf16, tag="pt2")
                    nc.tensor.transpose(
                        pt2[0:fw, 0:TP], accv[:, r, f0:f0 + fw],
                        identb[0:TP, 0:TP],
                    )
                    nc.scalar.copy(
                        yv[0:fw, r:r + TAU * (TP - 1) + 1:TAU, comp],
                        pt2[0:fw, 0:TP],
                    )
    nc.gpsimd.dma_start(
        y_ap[0:256].rearrange("(b f) t c -> f b (t c)", b=2),
        yo01.rearrange("f (b w) -> f b w", b=2),
    )
    nc.gpsimd.dma_start(
        y_ap[256:257].rearrange("f t c -> f (t c)"), yo2[:]
    )


def _build():
    if "nc" in _CACHE:
        return _CACHE["nc"]
    from contextlib import ExitStack
    from concourse import bacc, mybir
    import concourse.tile as tile

    f32 = mybir.dt.float32
    nc = bacc.Bacc("TRN2", target_bir_lowering=False, debug=False, num_devices=B)
    m_d = nc.dram_tensor("m", (C, T, F), f32, kind="ExternalInput")
    x_d = nc.dram_tensor("x", (F, T, 2), f32, kind="ExternalInput")
    id_d = nc.dram_tensor("ident", (128, 128), f32, kind="ExternalInput")
    y_d = nc.dram_tensor("y", (F, T, 2), f32, kind="ExternalOutput")

    with tile.TileContext(nc) as tc:
        with ExitStack() as ctx:
            _emit(ctx, tc, m_d.ap(), x_d.ap(), id_d.ap(), y_d.ap())
    nc.compile()
    _CACHE["nc"] = nc
    return nc


def _in_maps(m, x):
    ident = np.eye(128, dtype=np.float32)
    return [
        {"m": np.ascontiguousarray(m[b]), "x": np.ascontiguousarray(x[b]),
         "ident": ident}
        for b in range(B)
    ]


def kernel(m, x, v, _trace=False):
    from concourse import bass_utils

    m = np.asarray(m, dtype=np.float32)
    x = np.asarray(x, dtype=np.float32)
    nc = _build()
    res = bass_utils.run_bass_kernel_spmd(
        nc, _in_maps(m, x), core_ids=list(range(B)), trace=_trace
    )
    kernel.last_results = res
    y = np.stack(
        [np.asarray(res.results[b]["y"], dtype=np.float32) for b in range(B)],
        axis=0,
    )
    return y



# revision 8
# speedup vs baseline: 1.3117x; 1.3117x over previous
"""Trainium2 Bass kernel for nn_CCM: per-pixel complex 3x3 conv mask.

Math (per batch element b, sharded 1 batch element per NeuronCore):
  y[t,f] = sum_{n=0..8} A_n[t,f] * X[t+i(n)-2, f+j(n)-1]   (complex)
with A_n = m_n + w * m_{9+n} + conj(w) * m_{18+n}, w = -1/2 + i*sqrt(3)/2:
  Ar_n = m_n - 0.5*(m_{9+n} + m_{18+n})
  Ai_n = s * (m_{9+n} - m_{18+n}),  s = sqrt(3)/2
X = xr + i*xi, zero padded (causal in t: 2 top; symmetric in f: 1,1).

v2 design (from trace analysis of the v1 kernel):
- v1 was DVE-bound (113.9us busy) while DMA loads need only ~84us. The 36
  accumulation adds are moved OFF the DVE: per-tap products go to PSUM via
  identity-weight matmuls on the idle PE (PSUM accumulates in fp32, which
  also improves precision over bf16 accumulators).
- scalar_tensor_tensor fuses the -0.5 / +-sqrt(3)/2 basis scalings into the
  prep/product ops, freeing the Scalar engine for drains + output copies.
- PSUM capacity is 4096 fp32/partition but the output needs 2*2056: the
  f=256 column is accumulated in SBUF by tiny strided DVE adds reading the
  full-width product tiles (taps with j==2 contribute zero there and are
  skipped).
- m loads are merged: one casting SWDGE DMA per tap for the (m9+n, m18+n)
  pair and one for m_n (18 descgens instead of 27).
- Tap 8 runs in 4 tau-quarters, each flowing products -> PE accumulate ->
  ACT drain -> PE transpose -> ACT copy -> store so the tail is short.
- PSUM banks are time-shared via same-tag tile reuse: head x-transposes ->
  accumulators -> tail output transposes.
"""

import sys
import numpy as np

sys.path.insert(0, "/opt/trn_rl_repo")

B = 8
C = 27
T = 1000
F = 257
TP = 125          # partitions
TAU = 8           # t = 8*p + tau
NS = 10           # slots in x planes: tau in [-2, 8)
SROW = 260        # x plane slot row width (elements)
MW = TAU * F      # 2056: m / prod tile width (flat, rows of 257)
AW = TAU * 256    # 2048: psum accumulator width (rows of 256)
PLW = NS * SROW   # 2600: x plane width
SQ3H = float(np.sqrt(3.0) / 2.0)

_CACHE = {}


def _emit(ctx, tc, m_ap, x_ap, id_ap, y_ap):
    import concourse.mybir as mybir

    nc = tc.nc
    f32 = mybir.dt.float32
    bf16 = mybir.dt.bfloat16
    mult = mybir.AluOpType.mult
    addop = mybir.AluOpType.add
    FCS = [(0, 128), (128, 128), (256, 1)]   # f chunks for transposes
    SLOT_GROUPS = [(0, 4), (4, 4), (8, 2)]   # batches of slots per psum tile

    const = ctx.enter_context(tc.tile_pool(name="const", bufs=1))
    mcpool = ctx.enter_context(tc.tile_pool(name="mc", bufs=1))
    prep = ctx.enter_context(tc.tile_pool(name="prep", bufs=2))
    prod = ctx.enter_context(tc.tile_pool(name="prod", bufs=6))
    yop = ctx.enter_context(tc.tile_pool(name="yop", bufs=1))
    # One PSUM pool; the 8 banks rotate roles via tag reuse:
    # head ptg transposes -> accr/acci accumulators -> tail pt2 transposes.
    psum = ctx.enter_context(tc.tile_pool(name="psum", bufs=1, space="PSUM"))
    BTAGS = [f"bank{i}" for i in range(8)]

    # ---- ident via HWDGE (sync), x + m via SWDGE casting DMAs on gpsimd.
    ident = const.tile([128, 128], f32, tag="ident")
    nc.sync.dma_start(ident[:], id_ap)
    identb = const.tile([128, 128], bf16, tag="identb")
    nc.scalar.copy(identb[:], ident[:])

    mp = {}
    ms = {}

    def load_mpair(n):
        t = mcpool.tile([TP, 2 * MW], bf16, tag=f"mp{n}", name=f"mp{n}")
        nc.gpsimd.dma_start(
            t.rearrange("p (c w) -> p c w", c=2),
            m_ap[9 + n:19 + n:9].rearrange("c (p t) f -> p c (t f)", p=TP),
        )
        mp[n] = t

    def load_msingle(n):
        t = mcpool.tile([TP, MW], bf16, tag=f"ms{n}", name=f"ms{n}")
        nc.gpsimd.dma_start(
            t[:], m_ap[n].rearrange("(p t) f -> p (t f)", p=TP)
        )
        ms[n] = t

    xns = []
    for ci, (f0, fw) in enumerate(FCS):
        xn = const.tile([fw, (T + 2) * 2], bf16, tag=f"xn{f0}")
        nc.vector.memset(xn[:, 0:4], 0.0)
        nc.gpsimd.dma_start(
            xn[:, 4:], x_ap[f0:f0 + fw].rearrange("f t c -> f (t c)")
        )
        xns.append(xn)
        if ci == 0:
            load_mpair(0)
        elif ci == 1:
            load_msingle(0)
    for n in range(1, 9):
        load_mpair(n)
        load_msingle(n)

    # ---- x planes (bf16): f origin at col 1, zero pads at cols 0, 258, 259
    planes = {}
    for nm in ("xr", "xi"):
        p = const.tile([TP, PLW], bf16, tag=nm, name=nm)
        pv = p.rearrange("p (s w) -> p s w", w=SROW)
        nc.vector.memset(pv[:, :, 0:1], 0.0)
        nc.vector.memset(pv[:, :, 258:260], 0.0)
        planes[nm] = p

    # sideband (f=256) accumulators, one per complex component
    sbr = const.tile([TP, TAU], bf16, tag="sbr")
    sbi = const.tile([TP, TAU], bf16, tag="sbi")
    nc.vector.memset(sbr[:], 0.0)
    nc.vector.memset(sbi[:], 0.0)

    # ---- transpose x into the planes: PE (into psum bank slots) + copies.
    # Slot group (8,2) last: it is only read from tap i=1 (n>=3) onward.
    bank_rr = 0
    for (g0, gn) in SLOT_GROUPS:
        for ci, (f0, fw) in enumerate(FCS):
            xn3 = xns[ci].rearrange("f (t c) -> f t c", c=2)
            for q, nm in enumerate(("xr", "xi")):
                pA = planes[nm].rearrange("p (s w) -> p s w", w=SROW)
                ptg = psum.tile(
                    [TP, 512], bf16, tag=BTAGS[bank_rr % 8], name="ptg",
                    padded_shape=[128, 1024],
                )
                bank_rr += 1
                for u in range(gn):
                    ts = g0 + u
                    nc.tensor.transpose(
                        ptg[0:TP, 128 * u:128 * u + fw],
                        xn3[0:fw, ts:ts + TAU * (TP - 1) + 1:TAU, q],
                        identb[0:fw, 0:fw],
                    )
                src = ptg.rearrange("p (u w) -> p u w", w=128)[0:TP, 0:gn, 0:fw]
                dst = pA[:, g0:g0 + gn, 1 + f0:1 + f0 + fw]
                nc.scalar.copy(dst, src)

    # ---- PSUM accumulators: accr rows of 256 in banks 0-3, acci in 4-7.
    # 512 fp32 = one bank = two tau rows.
    accr_c = [
        psum.tile([TP, 512], f32, tag=BTAGS[c], name=f"accr{c}",
                  padded_shape=[128, 512])
        for c in range(4)
    ]
    acci_c = [
        psum.tile([TP, 512], f32, tag=BTAGS[4 + c], name=f"acci{c}",
                  padded_shape=[128, 512])
        for c in range(4)
    ]

    # ---- tap loop. DVE: prep (t1, d, ar with fused -0.5) + 4 full-width
    # products (s folded in via scalar_tensor_tensor). PE: accumulate each
    # product's main 2048 cols into PSUM via identity matmuls. Tiny strided
    # DVE adds accumulate the f=256 column (zero for j==2 taps).
    idw = identb[0:TP, 0:TP]

    def prep_tap(n):
        pair = mp[n]
        m9 = pair[:, 0:MW]
        m18 = pair[:, MW:2 * MW]
        t1 = prep.tile([TP, MW], bf16, tag="t1", name="t1")
        nc.vector.tensor_add(t1[:], m9, m18)
        d = prep.tile([TP, MW], bf16, tag="d", name="d")
        nc.vector.tensor_sub(d[:], m9, m18)
        ar = prep.tile([TP, MW], bf16, tag="ar", name="ar")
        nc.vector.scalar_tensor_tensor(ar[:], t1[:], -0.5, ms[n][:], mult, addop)
        return ar, d

    def products(n, ar, d, r0, rn):
        """Emit the 4 product ops for tau rows [r0, r0+rn); returns tiles."""
        i, j = divmod(n, 3)
        xrv = planes["xr"].rearrange("p (s w) -> p s w", w=SROW)[
            :, i + r0:i + r0 + rn, j:j + F]
        xiv = planes["xi"].rearrange("p (s w) -> p s w", w=SROW)[
            :, i + r0:i + r0 + rn, j:j + F]
        a8 = ar.rearrange("p (r w) -> p r w", w=F)[:, r0:r0 + rn, :]
        d8 = d.rearrange("p (r w) -> p r w", w=F)[:, r0:r0 + rn, :]
        w = rn * F
        p0 = prod.tile([TP, w], bf16, tag="P", name="p0")
        p1 = prod.tile([TP, w], bf16, tag="P", name="p1")
        p2 = prod.tile([TP, w], bf16, tag="P", name="p2")
        p3 = prod.tile([TP, w], bf16, tag="P", name="p3")
        p08 = p0.rearrange("p (r w) -> p r w", w=F)
        p18 = p1.rearrange("p (r w) -> p r w", w=F)
        p28 = p2.rearrange("p (r w) -> p r w", w=F)
        p38 = p3.rearrange("p (r w) -> p r w", w=F)
        nc.vector.tensor_mul(p08, a8, xrv)
        nc.vector.tensor_mul(p18, a8, xiv)
        nc.vector.scalar_tensor_tensor(p28, d8, -SQ3H, xiv, mult, mult)
        nc.vector.scalar_tensor_tensor(p38, d8, SQ3H, xrv, mult, mult)
        return p0, p1, p2, p3

    def accum_main(n, tiles, r0, rn, last):
        """PE-accumulate rows [r0, r0+rn) of the product tiles into PSUM."""
        p0, p1, p2, p3 = tiles
        first = n == 0
        for c in range(r0 // 2, (r0 + rn) // 2):
            lo = c * 2 - r0
            for acc, pa, pb in ((accr_c[c], p0, p2), (acci_c[c], p1, p3)):
                for k, pt in enumerate((pa, pb)):
                    pv = pt.rearrange("p (r w) -> p r w", w=F)[
                        :, lo:lo + 2, 0:256]
                    nc.tensor.matmul(
                        acc[:], idw, pv,
                        start=(first and k == 0),
                        stop=(last and k == 1),
                    )

    def accum_sb(n, tiles, r0, rn):
        if n % 3 == 2:
            return
        p0, p1, p2, p3 = tiles
        for acc, pa, pb in ((sbr, p0, p2), (sbi, p1, p3)):
            for pt in (pa, pb):
                pv = pt.rearrange("p (r w) -> p r w", w=F)[:, 0:rn, 256]
                nc.vector.tensor_add(acc[:, r0:r0 + rn], acc[:, r0:r0 + rn], pv)

    # drained (bf16) accumulators for the output transposes
    accr_s = const.tile([TP, AW], bf16, tag="accr_s")
    acci_s = const.tile([TP, AW], bf16, tag="acci_s")

    yo01 = yop.tile([128, 2 * T * 2], bf16, tag="yo01", name="yo01")
    yo2 = yop.tile([1, T * 2], bf16, tag="yo2", name="yo2")
    yviews = [
        yo01[:, 0:T * 2].rearrange("f (t c) -> f t c", c=2),
        yo01[:, T * 2:].rearrange("f (t c) -> f t c", c=2),
        yo2.rearrange("f (t c) -> f t c", c=2),
    ]

    def drain_chunk(c):
        nc.scalar.copy(accr_s[:, 512 * c:512 * (c + 1)], accr_c[c][:])
        nc.scalar.copy(acci_s[:, 512 * c:512 * (c + 1)], acci_c[c][:])

    def out_quarter(c):
        """Transpose + copy tau rows 2c, 2c+1 of both comps, all f chunks."""
        accvs = [
            accr_s.rearrange("p (r w) -> p r w", w=256),
            acci_s.rearrange("p (r w) -> p r w", w=256),
        ]
        sbs = [sbr, sbi]
        for ci, (f0, fw) in enumerate(FCS):
            yv = yviews[ci]
            for comp in (0, 1):
                for r in (2 * c, 2 * c + 1):
                    # only banks c (accr) and 4+c (acci) are retired by
                    # drain_chunk(c); later chunks are still accumulating.
                    pt2 = psum.tile(
                        [128, TP], bf16, tag=BTAGS[c + 4 * (r % 2)],
                        name="pt2", padded_shape=[128, 1024],
                    )
                    if ci < 2:
                        src = accvs[comp][:, r, f0:f0 + fw]
                    else:
                        src = sbs[comp][:, r:r + 1]
                    nc.tensor.transpose(pt2[0:fw, 0:TP], src, idw)
                    nc.scalar.copy(
                        yv[0:fw, r:r + TAU * (TP - 1) + 1:TAU, comp],
                        pt2[0:fw, 0:TP],
                    )

    for n in range(C // 3):
        ar, d = prep_tap(n)
        if n < 8:
            tiles = products(n, ar, d, 0, TAU)
            accum_main(n, tiles, 0, TAU, last=False)
            accum_sb(n, tiles, 0, TAU)
        else:
            for c in range(4):
                tiles = products(n, ar, d, 2 * c, 2)
                accum_main(n, tiles, 2 * c, 2, last=True)
                accum_sb(n, tiles, 2 * c, 2)
                drain_chunk(c)
                out_quarter(c)

    # ---- stores: split per f-chunk so each can fire as its copies finish.
    nc.gpsimd.dma_start(
        y_ap[0:128].rearrange("f t c -> f (t c)"), yo01[:, 0:T * 2]
    )
    nc.gpsimd.dma_start(
        y_ap[128:256].rearrange("f t c -> f (t c)"), yo01[0:128, T * 2:]
    )
    nc.gpsimd.dma_start(
        y_ap[256:257].rearrange("f t c -> f (t c)"), yo2[:]
    )


def _build():
    if "nc" in _CACHE:
        return _CACHE["nc"]
    from contextlib import ExitStack
    from concourse import bacc, mybir
    import concourse.tile as tile

    f32 = mybir.dt.float32
    nc = bacc.Bacc("TRN2", target_bir_lowering=False, debug=False, num_devices=B)
    m_d = nc.dram_tensor("m", (C, T, F), f32, kind="ExternalInput")
    x_d = nc.dram_tensor("x", (F, T, 2), f32, kind="ExternalInput")
    id_d = nc.dram_tensor("ident", (128, 128), f32, kind="ExternalInput")
    y_d = nc.dram_tensor("y", (F, T, 2), f32, kind="ExternalOutput")

    with tile.TileContext(nc) as tc:
        with ExitStack() as ctx:
            _emit(ctx, tc, m_d.ap(), x_d.ap(), id_d.ap(), y_d.ap())
    nc.compile()
    _CACHE["nc"] = nc
    return nc


def _in_maps(m, x):
    ident = np.eye(128, dtype=np.float32)
    return [
        {"m": np.ascontiguousarray(m[b]), "x": np.ascontiguousarray(x[b]),
         "ident": ident}
        for b in range(B)
    ]


def kernel(m, x, v, _trace=False):
    from concourse import bass_utils

    m = np.asarray(m, dtype=np.float32)
    x = np.asarray(x, dtype=np.float32)
    nc = _build()
    res = bass_utils.run_bass_kernel_spmd(
        nc, _in_maps(m, x), core_ids=list(range(B)), trace=_trace
    )
    kernel.last_results = res
    y = np.stack(
        [np.asarray(res.results[b]["y"], dtype=np.float32) for b in range(B)],
        axis=0,
    )
    return y


# revision 9
# speedup vs baseline: 1.4987x; 1.1426x over previous
"""Trainium2 Bass kernel for nn_CCM: per-pixel complex 3x3 conv mask.

Math (per batch element b, sharded 1 batch element per NeuronCore):
  y[t,f] = sum_{n=0..8} A_n[t,f] * X[t+i(n)-2, f+j(n)-1]   (complex)
with A_n = m_n + w * m_{9+n} + conj(w) * m_{18+n}, w = -1/2 + i*sqrt(3)/2:
  Ar_n = m_n - 0.5*(m_{9+n} + m_{18+n})
  Ai_n = s * (m_{9+n} - m_{18+n}),  s = sqrt(3)/2
X = xr + i*xi, zero padded (causal in t: 2 top; symmetric in f: 1,1).

v3 design (from trace analysis of v1/v2):
- v1 was DVE-bound (113.9us busy; DMA loads need only ~84us). The 36
  accumulation adds are moved OFF the DVE: per-tap products accumulate into
  PSUM fp32 via identity-weight matmuls on the otherwise idle PE (this also
  improves precision over v1's bf16 accumulators).
- scalar_tensor_tensor measured at 1x DVE mode (no 2x uop) - avoided.
  Instead the +-s basis scale is folded into two extra SCALED x planes
  (xrs=+s*xr, xin=-s*xi) built by whole-slot-group DVE tensor_scalar ops
  (4x mode, ~330ns each); products are then plain 2x tensor_tensor muls.
  th=-0.5*t1 also runs as DVE tensor_scalar at 4x.
- PSUM holds 4096 fp32/partition but the output needs 2*2056: the f=256
  column is accumulated in SBUF by tiny strided DVE adds reading the
  full-width product tiles (taps with j==2 contribute zero there; skipped).
- m loads are merged per tap: one casting SWDGE DMA for the (m9+n, m18+n)
  pair and one for m_n. identb loads via a casting SWDGE DMA first so the
  head x-transposes start at ~4us (v2 lost ~10us to sync-DMA + ACT here).
- Tap 8 runs in 4 tau-quarters, each flowing products -> PE accumulate ->
  ACT drain -> PE transpose -> DVE/ACT copies -> store; output copies are
  split across DVE (2x psum reads) and ACT to shorten the tail.
- PSUM banks are time-shared via same-tag tile reuse: head x-transposes ->
  accumulators -> tail output transposes (quarter c only reuses banks c and
  4+c, which drain_chunk(c) has retired).
"""

import sys
import numpy as np

sys.path.insert(0, "/opt/trn_rl_repo")

B = 8
C = 27
T = 1000
F = 257
TP = 125          # partitions
TAU = 8           # t = 8*p + tau
NS = 10           # slots in x planes: tau in [-2, 8)
SROW = 260        # x plane slot row width (elements)
MW = TAU * F      # 2056: m / prod tile width (flat, rows of 257)
AW = TAU * 256    # 2048: psum accumulator width (rows of 256)
PLW = NS * SROW   # 2600: x plane width
SQ3H = float(np.sqrt(3.0) / 2.0)

_CACHE = {}


def _emit(ctx, tc, m_ap, x_ap, id_ap, y_ap):
    import concourse.mybir as mybir

    nc = tc.nc
    f32 = mybir.dt.float32
    bf16 = mybir.dt.bfloat16
    FCS = [(0, 128), (128, 128), (256, 1)]   # f chunks for transposes
    SLOT_GROUPS = [(0, 4), (4, 4), (8, 2)]   # batches of slots per psum tile

    const = ctx.enter_context(tc.tile_pool(name="const", bufs=1))
    mcpool = ctx.enter_context(tc.tile_pool(name="mc", bufs=1))
    prep = ctx.enter_context(tc.tile_pool(name="prep", bufs=1))
    prod = ctx.enter_context(tc.tile_pool(name="prod", bufs=6))
    yop = ctx.enter_context(tc.tile_pool(name="yop", bufs=1))
    # One PSUM pool; the 8 banks rotate roles via tag reuse:
    # head ptg transposes -> accr/acci accumulators -> tail pt2 transposes.
    psum = ctx.enter_context(tc.tile_pool(name="psum", bufs=1, space="PSUM"))
    BTAGS = [f"bank{i}" for i in range(8)]

    # ---- identb first (casting SWDGE, tiny), then x, then m per-tap pairs.
    identb = const.tile([128, 128], bf16, tag="identb")
    nc.gpsimd.dma_start(identb[:], id_ap)

    mp = {}
    ms = {}

    def load_mpair(n):
        t = mcpool.tile([TP, 2 * MW], bf16, tag=f"mp{n}", name=f"mp{n}")
        nc.gpsimd.dma_start(
            t.rearrange("p (c w) -> p c w", c=2),
            m_ap[9 + n:19 + n:9].rearrange("c (p t) f -> p c (t f)", p=TP),
        )
        mp[n] = t

    def load_msingle(n):
        t = mcpool.tile([TP, MW], bf16, tag=f"ms{n}", name=f"ms{n}")
        nc.gpsimd.dma_start(
            t[:], m_ap[n].rearrange("(p t) f -> p (t f)", p=TP)
        )
        ms[n] = t

    xns = []
    for ci, (f0, fw) in enumerate(FCS):
        xn = const.tile([fw, (T + 2) * 2], bf16, tag=f"xn{f0}")
        nc.vector.memset(xn[:, 0:4], 0.0)
        nc.gpsimd.dma_start(
            xn[:, 4:], x_ap[f0:f0 + fw].rearrange("f t c -> f (t c)")
        )
        xns.append(xn)
    load_mpair(0)
    load_msingle(0)
    for n in range(1, 9):
        load_mpair(n)
        load_msingle(n)

    # ---- x planes (bf16): f origin at col 1, zero pads at cols 0, 258, 259.
    # xr/xi are plain; xrs = +s*xr and xin = -s*xi carry the basis scale so
    # the d-products stay plain 2x tensor_tensor muls.
    planes = {}
    for nm in ("xr", "xi", "xrs", "xin"):
        p = const.tile([TP, PLW], bf16, tag=nm, name=nm)
        if nm in ("xr", "xi"):
            pv = p.rearrange("p (s w) -> p s w", w=SROW)
            nc.vector.memset(pv[:, :, 0:1], 0.0)
            nc.vector.memset(pv[:, :, 258:260], 0.0)
        planes[nm] = p

    # sideband (f=256) accumulators, one per complex component
    sbr = const.tile([TP, TAU], bf16, tag="sbr")
    sbi = const.tile([TP, TAU], bf16, tag="sbi")
    nc.vector.memset(sbr[:], 0.0)
    nc.vector.memset(sbi[:], 0.0)

    # ---- transpose x into the planes: PE (into psum bank slots) + ACT
    # copies. Slot group (8,2) last: only read from tap i=1 (n>=3) onward.
    bank_rr = 0
    for (g0, gn) in SLOT_GROUPS:
        for ci, (f0, fw) in enumerate(FCS):
            xn3 = xns[ci].rearrange("f (t c) -> f t c", c=2)
            for q, nm in enumerate(("xr", "xi")):
                pA = planes[nm].rearrange("p (s w) -> p s w", w=SROW)
                ptg = psum.tile(
                    [TP, 512], bf16, tag=BTAGS[bank_rr % 8], name="ptg",
                    padded_shape=[128, 1024],
                )
                bank_rr += 1
                for u in range(gn):
                    ts = g0 + u
                    nc.tensor.transpose(
                        ptg[0:TP, 128 * u:128 * u + fw],
                        xn3[0:fw, ts:ts + TAU * (TP - 1) + 1:TAU, q],
                        identb[0:fw, 0:fw],
                    )
                src = ptg.rearrange("p (u w) -> p u w", w=128)[0:TP, 0:gn, 0:fw]
                dst = pA[:, g0:g0 + gn, 1 + f0:1 + f0 + fw]
                nc.scalar.copy(dst, src)

    def scale_group(g0, gn):
        """Build the scaled-plane slots [g0, g0+gn) with 4x tensor_scalar."""
        for src_nm, dst_nm, sc in (("xr", "xrs", SQ3H), ("xi", "xin", -SQ3H)):
            sv = planes[src_nm].rearrange("p (s w) -> p s w", w=SROW)
            dv = planes[dst_nm].rearrange("p (s w) -> p s w", w=SROW)
            nc.vector.tensor_scalar_mul(
                dv[:, g0:g0 + gn, :], sv[:, g0:g0 + gn, :], sc
            )

    # ---- PSUM accumulators: accr rows of 256 in banks 0-3, acci in 4-7.
    # 512 fp32 = one bank = two tau rows.
    accr_c = [
        psum.tile([TP, 512], f32, tag=BTAGS[c], name=f"accr{c}",
                  padded_shape=[128, 512])
        for c in range(4)
    ]
    acci_c = [
        psum.tile([TP, 512], f32, tag=BTAGS[4 + c], name=f"acci{c}",
                  padded_shape=[128, 512])
        for c in range(4)
    ]

    idw = identb[0:TP, 0:TP]

    def prep_tap(n):
        pair = mp[n]
        m9 = pair[:, 0:MW]
        m18 = pair[:, MW:2 * MW]
        t1 = prep.tile([TP, MW], bf16, tag="t1", name="t1")
        nc.vector.tensor_add(t1[:], m9, m18)
        d = prep.tile([TP, MW], bf16, tag="d", name="d")
        nc.vector.tensor_sub(d[:], m9, m18)
        th = prep.tile([TP, MW], bf16, tag="th", name="th")
        nc.vector.tensor_scalar_mul(th[:], t1[:], -0.5)
        ar = prep.tile([TP, MW], bf16, tag="ar", name="ar")
        nc.vector.tensor_add(ar[:], th[:], ms[n][:])
        return ar, d

    def products(n, ar, d, r0, rn):
        """Emit the 4 product ops for tau rows [r0, r0+rn); returns tiles."""
        i, j = divmod(n, 3)

        def xv(nm):
            return planes[nm].rearrange("p (s w) -> p s w", w=SROW)[
                :, i + r0:i + r0 + rn, j:j + F]

        a8 = ar.rearrange("p (r w) -> p r w", w=F)[:, r0:r0 + rn, :]
        d8 = d.rearrange("p (r w) -> p r w", w=F)[:, r0:r0 + rn, :]
        w = rn * F
        p0 = prod.tile([TP, w], bf16, tag="P", name="p0")
        p1 = prod.tile([TP, w], bf16, tag="P", name="p1")
        p2 = prod.tile([TP, w], bf16, tag="P", name="p2")
        p3 = prod.tile([TP, w], bf16, tag="P", name="p3")
        nc.vector.tensor_mul(p0.rearrange("p (r w) -> p r w", w=F), a8, xv("xr"))
        nc.vector.tensor_mul(p1.rearrange("p (r w) -> p r w", w=F), a8, xv("xi"))
        nc.vector.tensor_mul(p2.rearrange("p (r w) -> p r w", w=F), d8, xv("xin"))
        nc.vector.tensor_mul(p3.rearrange("p (r w) -> p r w", w=F), d8, xv("xrs"))
        return p0, p1, p2, p3

    def accum_main(n, tiles, r0, rn, last):
        """PE-accumulate rows [r0, r0+rn) of the product tiles into PSUM."""
        p0, p1, p2, p3 = tiles
        first = n == 0
        for c in range(r0 // 2, (r0 + rn) // 2):
            lo = c * 2 - r0
            for acc, pa, pb in ((accr_c[c], p0, p2), (acci_c[c], p1, p3)):
                for k, pt in enumerate((pa, pb)):
                    pv = pt.rearrange("p (r w) -> p r w", w=F)[
                        :, lo:lo + 2, 0:256]
                    nc.tensor.matmul(
                        acc[:], idw, pv,
                        start=(first and k == 0),
                        stop=(last and k == 1),
                    )

    def accum_sb(n, tiles, r0, rn):
        if n % 3 == 2:
            return
        p0, p1, p2, p3 = tiles
        for acc, pa, pb in ((sbr, p0, p2), (sbi, p1, p3)):
            for pt in (pa, pb):
                pv = pt.rearrange("p (r w) -> p r w", w=F)[:, 0:rn, 256]
                nc.vector.tensor_add(acc[:, r0:r0 + rn], acc[:, r0:r0 + rn], pv)

    # drained (bf16) accumulators for the output transposes
    accr_s = const.tile([TP, AW], bf16, tag="accr_s")
    acci_s = const.tile([TP, AW], bf16, tag="acci_s")

    yo01 = yop.tile([128, 2 * T * 2], bf16, tag="yo01", name="yo01")
    yo2 = yop.tile([1, T * 2], bf16, tag="yo2", name="yo2")
    yviews = [
        yo01[:, 0:T * 2].rearrange("f (t c) -> f t c", c=2),
        yo01[:, T * 2:].rearrange("f (t c) -> f t c", c=2),
        yo2.rearrange("f (t c) -> f t c", c=2),
    ]

    def drain_chunk(c):
        nc.scalar.copy(accr_s[:, 512 * c:512 * (c + 1)], accr_c[c][:])
        nc.scalar.copy(acci_s[:, 512 * c:512 * (c + 1)], acci_c[c][:])

    def out_quarter(c):
        """Transpose + copy tau rows 2c, 2c+1 of both comps, all f chunks."""
        accvs = [
            accr_s.rearrange("p (r w) -> p r w", w=256),
            acci_s.rearrange("p (r w) -> p r w", w=256),
        ]
        sbs = [sbr, sbi]
        for ci, (f0, fw) in enumerate(FCS):
            yv = yviews[ci]
            for comp in (0, 1):
                for r in (2 * c, 2 * c + 1):
                    # only banks c (accr) and 4+c (acci) are retired by
                    # drain_chunk(c); later chunks are still accumulating.
                    pt2 = psum.tile(
                        [128, TP], bf16, tag=BTAGS[c + 4 * (r % 2)],
                        name="pt2", padded_shape=[128, 1024],
                    )
                    if ci < 2:
                        src = accvs[comp][:, r, f0:f0 + fw]
                    else:
                        src = sbs[comp][:, r:r + 1]
                    nc.tensor.transpose(pt2[0:fw, 0:TP], src, idw)
                    dst = yv[0:fw, r:r + TAU * (TP - 1) + 1:TAU, comp]
                    if ci == 0:
                        # DVE psum-read copies run at 2x and the DVE is idle
                        # by the tail; ACT takes the rest.
                        nc.vector.tensor_copy(dst, pt2[0:fw, 0:TP])
                    else:
                        nc.scalar.copy(dst, pt2[0:fw, 0:TP])

    for n in range(C // 3):
        ar, d = prep_tap(n)
        if n == 0:
            scale_group(0, 4)
            scale_group(4, 4)
        elif n == 2:
            scale_group(8, 2)
        if n < 8:
            tiles = products(n, ar, d, 0, TAU)
            accum_main(n, tiles, 0, TAU, last=False)
            accum_sb(n, tiles, 0, TAU)
        else:
            for c in range(4):
                tiles = products(n, ar, d, 2 * c, 2)
                accum_main(n, tiles, 2 * c, 2, last=True)
                accum_sb(n, tiles, 2 * c, 2)
                drain_chunk(c)
                out_quarter(c)

    # ---- stores: split per f-chunk so each can fire as its copies finish.
    nc.gpsimd.dma_start(
        y_ap[0:128].rearrange("f t c -> f (t c)"), yo01[:, 0:T * 2]
    )
    nc.gpsimd.dma_start(
        y_ap[128:256].rearrange("f t c -> f (t c)"), yo01[0:128, T * 2:]
    )
    nc.gpsimd.dma_start(
        y_ap[256:257].rearrange("f t c -> f (t c)"), yo2[:]
    )


def _build():
    if "nc" in _CACHE:
        return _CACHE["nc"]
    from contextlib import ExitStack
    from concourse import bacc, mybir
    import concourse.tile as tile

    f32 = mybir.dt.float32
    nc = bacc.Bacc("TRN2", target_bir_lowering=False, debug=False, num_devices=B)
    m_d = nc.dram_tensor("m", (C, T, F), f32, kind="ExternalInput")
    x_d = nc.dram_tensor("x", (F, T, 2), f32, kind="ExternalInput")
    id_d = nc.dram_tensor("ident", (128, 128), f32, kind="ExternalInput")
    y_d = nc.dram_tensor("y", (F, T, 2), f32, kind="ExternalOutput")

    with tile.TileContext(nc) as tc:
        with ExitStack() as ctx:
            _emit(ctx, tc, m_d.ap(), x_d.ap(), id_d.ap(), y_d.ap())
    nc.compile()
    _CACHE["nc"] = nc
    return nc


def _in_maps(m, x):
    ident = np.eye(128, dtype=np.float32)
    return [
        {"m": np.ascontiguousarray(m[b]), "x": np.ascontiguousarray(x[b]),
         "ident": ident}
        for b in range(B)
    ]


def kernel(m, x, v, _trace=False):
    from concourse import bass_utils

    m = np.asarray(m, dtype=np.float32)
    x = np.asarray(x, dtype=np.float32)
    nc = _build()
    res = bass_utils.run_bass_kernel_spmd(
        nc, _in_maps(m, x), core_ids=list(range(B)), trace=_trace
    )
    kernel.last_results = res
    y = np.stack(
        [np.asarray(res.results[b]["y"], dtype=np.float32) for b in range(B)],
        axis=0,
    )
    return y


# revision 14
# speedup vs baseline: 1.4991x; 1.0002x over previous
"""Trainium2 Bass kernel for nn_CCM: per-pixel complex 3x3 conv mask.

Math (per batch element b, sharded 1 batch element per NeuronCore):
  y[t,f] = sum_{n=0..8} A_n[t,f] * X[t+i(n)-2, f+j(n)-1]   (complex)
with A_n = m_n + w * m_{9+n} + conj(w) * m_{18+n}, w = -1/2 + i*sqrt(3)/2:
  Ar_n = m_n - 0.5*(m_{9+n} + m_{18+n})
  Ai_n = s * (m_{9+n} - m_{18+n}),  s = sqrt(3)/2
X = xr + i*xi, zero padded (causal in t: 2 top; symmetric in f: 1,1).

v4 design (trace-driven, from v1-v3):
- The kernel is jointly limited by the 29.8MB fp32 load stream (~84us at
  358GB/s) and the DVE. Everything else is arranged to keep both saturated:
- Per-tap products accumulate into PSUM fp32 via identity-weight matmuls on
  the PE (removes all accumulation adds from the DVE; better precision than
  bf16 accumulators).
- The +-s basis scale lives in two extra SCALED x planes (xrs=+s*xr,
  xin=-s*xi); th=-0.5*(m9+m18) runs on ACT with one-tap lookahead so the
  DVE never waits. DVE per tap: t1, d, ar + 4 plain 2x tensor_tensor muls.
- x stages via HWDGE (sync) as fp32 so the SWDGE m-stream starts ~4us
  earlier; x transposes run in fp32, plane copies cast to bf16.
- PSUM holds 4096 fp32/partition but the output needs 2*2056 so the f=256
  column accumulates in SBUF via tiny strided DVE adds (j==2 taps are zero
  there and skipped); it stores via a direct scatter SWDGE cast DMA.
- Output: per (f-half, comp) ONE psum bank collects all 8 tau-row
  transposes (start=False accumulate-into-disjoint-elements), then ONE
  contiguous-source copy into the bf16 staging tile; casting SWDGE stores.
- Tap 8's m tiles load in tau-quarters so its products/dr ains/transposes
  pipeline with the last DMA arrivals.
- PSUM banks are time-shared via same-tag tile reuse: head x-transposes ->
  accumulators -> tail output collectors.
"""

import sys
import numpy as np

sys.path.insert(0, "/opt/trn_rl_repo")

B = 8
C = 27
T = 1000
F = 257
TP = 125          # partitions
TAU = 8           # t = 8*p + tau
NS = 10           # slots in x planes: tau in [-2, 8)
SROW = 260        # x plane slot row width (elements)
MW = TAU * F      # 2056: m / prod tile width (flat, rows of 257)
AW = TAU * 256    # 2048: psum accumulator width (rows of 256)
PLW = NS * SROW   # 2600: x plane width
QW = 2 * F        # 514: one tau-quarter of a flat m plane
SQ3H = float(np.sqrt(3.0) / 2.0)

_CACHE = {}


def _emit(ctx, tc, m_ap, x_ap, id_ap, y_ap):
    import concourse.mybir as mybir

    nc = tc.nc
    f32 = mybir.dt.float32
    bf16 = mybir.dt.bfloat16
    FCS = [(0, 128), (128, 128), (256, 1)]   # f chunks for transposes
    SLOT_GROUPS = [(0, 4), (4, 4), (8, 2)]   # batches of slots per psum tile

    const = ctx.enter_context(tc.tile_pool(name="const", bufs=1))
    mcpool = ctx.enter_context(tc.tile_pool(name="mc", bufs=1))
    xpool = tc.alloc_tile_pool(name="xstage", bufs=1)
    # One PSUM pool; the 8 banks rotate roles via tag reuse:
    # head ptg transposes -> accr/acci accumulators -> tail out collectors.
    psum = ctx.enter_context(tc.tile_pool(name="psum", bufs=1, space="PSUM"))
    BTAGS = [f"bank{i}" for i in range(8)]

    # ---- ident via HWDGE (sync, f32); identb cast on DVE in the idle head.
    ident = const.tile([128, 128], f32, tag="ident")
    nc.sync.dma_start(ident[:], id_ap)
    identb = const.tile([128, 128], bf16, tag="identb")

    # x staging via HWDGE in fp32: keeps the SWDGE queue free for m.
    xns = []
    for ci, (f0, fw) in enumerate(FCS):
        xn = xpool.tile([fw, (T + 2) * 2], f32, tag=f"xn{f0}", name=f"xn{f0}")
        nc.vector.memset(xn[:, 0:4], 0.0)
        nc.sync.dma_start(
            xn[:, 4:], x_ap[f0:f0 + fw].rearrange("f t c -> f (t c)")
        )
        xns.append(xn)

    # ---- m loads: casting SWDGE, one pair DMA + one single DMA per tap;
    # tap 8 loads in tau-quarters so the tail overlaps the last arrivals.
    mp = {}
    ms = {}

    def load_m(n):
        p = mcpool.tile([TP, 2 * MW], bf16, tag=f"mp{n}", name=f"mp{n}")
        s = mcpool.tile([TP, MW], bf16, tag=f"ms{n}", name=f"ms{n}")
        psrc = m_ap[9 + n:19 + n:9].rearrange("c (p t) f -> p c (t f)", p=TP)
        ssrc = m_ap[n].rearrange("(p t) f -> p (t f)", p=TP)
        pv = p.rearrange("p (c w) -> p c w", c=2)
        if n < 8:
            nc.gpsimd.dma_start(pv, psrc)
            nc.gpsimd.dma_start(s[:], ssrc)
        else:
            for q in range(4):
                sl = slice(QW * q, QW * (q + 1))
                nc.gpsimd.dma_start(pv[:, :, sl], psrc[:, :, sl])
                nc.gpsimd.dma_start(s[:, sl], ssrc[:, sl])
        mp[n], ms[n] = p, s

    for n in range(9):
        load_m(n)

    # identb cast early on the otherwise idle DVE
    nc.vector.tensor_copy(identb[:], ident[:])

    # ---- x planes (bf16): f origin at col 1, zero pads at cols 0, 258, 259.
    # xr/xi are plain; xrs = +s*xr and xin = -s*xi carry the basis scale.
    planes = {}
    for nm in ("xr", "xi", "xrs", "xin"):
        p = const.tile([TP, PLW], bf16, tag=nm, name=nm)
        if nm in ("xr", "xi"):
            pv = p.rearrange("p (s w) -> p s w", w=SROW)
            nc.vector.memset(pv[:, :, 0:1], 0.0)
            nc.vector.memset(pv[:, :, 258:260], 0.0)
        planes[nm] = p

    # sideband (f=256) accumulators, one per complex component
    sbr = const.tile([TP, TAU], bf16, tag="sbr")
    sbi = const.tile([TP, TAU], bf16, tag="sbi")
    nc.vector.memset(sbr[:], 0.0)
    nc.vector.memset(sbi[:], 0.0)

    # ---- transpose x into the planes (fp32 PE transposes into psum banks).
    # Copies cast fp32->bf16: slot groups g0 on the idle DVE, g1/g2 on ACT.
    # Scaled planes built per group right after (DVE for g0, ACT for rest).
    bank_rr = 0
    for gi, (g0, gn) in enumerate(SLOT_GROUPS):
        for ci, (f0, fw) in enumerate(FCS):
            xn3 = xns[ci].rearrange("f (t c) -> f t c", c=2)
            for q, nm in enumerate(("xr", "xi")):
                pA = planes[nm].rearrange("p (s w) -> p s w", w=SROW)
                ptg = psum.tile(
                    [TP, 512], f32, tag=BTAGS[bank_rr % 8], name="ptg",
                    padded_shape=[128, 512],
                )
                bank_rr += 1
                for u in range(gn):
                    ts = g0 + u
                    nc.tensor.transpose(
                        ptg[0:TP, 128 * u:128 * u + fw],
                        xn3[0:fw, ts:ts + TAU * (TP - 1) + 1:TAU, q],
                        ident[0:fw, 0:fw],
                    )
                src = ptg.rearrange("p (u w) -> p u w", w=128)[0:TP, 0:gn, 0:fw]
                dst = pA[:, g0:g0 + gn, 1 + f0:1 + f0 + fw]
                if gi == 0:
                    nc.vector.tensor_copy(dst, src)
                else:
                    nc.scalar.copy(dst, src)
        # scaled planes for this slot group
        for src_nm, dst_nm, sc in (("xr", "xrs", SQ3H), ("xi", "xin", -SQ3H)):
            sv = planes[src_nm].rearrange("p (s w) -> p s w", w=SROW)
            dv = planes[dst_nm].rearrange("p (s w) -> p s w", w=SROW)
            if gi == 0:
                nc.vector.tensor_scalar_mul(
                    dv[:, g0:g0 + gn, :], sv[:, g0:g0 + gn, :], sc)
            else:
                nc.scalar.mul(dv[:, g0:g0 + gn, :], sv[:, g0:g0 + gn, :], sc)
    xpool.release()

    prep = ctx.enter_context(tc.tile_pool(name="prep", bufs=2))
    prod = ctx.enter_context(tc.tile_pool(name="prod", bufs=4))
    yop = ctx.enter_context(tc.tile_pool(name="yop", bufs=1))

    # ---- PSUM accumulators: accr rows of 256 in banks 0-3, acci in 4-7.
    # 512 fp32 = one bank = two tau rows.
    accr_c = [
        psum.tile([TP, 512], f32, tag=BTAGS[c], name=f"accr{c}",
                  padded_shape=[128, 512])
        for c in range(4)
    ]
    acci_c = [
        psum.tile([TP, 512], f32, tag=BTAGS[4 + c], name=f"acci{c}",
                  padded_shape=[128, 512])
        for c in range(4)
    ]

    idw = identb[0:TP, 0:TP]

    def prep_td(n, r0=0, rn=TAU):
        """DVE half of prep: t1 = m9+m18, d = m9-m18 (tile-local rows)."""
        sl = slice(r0 * F, (r0 + rn) * F)
        m9 = mp[n][:, 0:MW][:, sl]
        m18 = mp[n][:, MW:2 * MW][:, sl]
        t1 = prep.tile([TP, rn * F], bf16, tag="t1", name="t1")
        nc.vector.tensor_add(t1[:], m9, m18)
        d = prep.tile([TP, rn * F], bf16, tag="d", name="d")
        nc.vector.tensor_sub(d[:], m9, m18)
        th = prep.tile([TP, rn * F], bf16, tag="th", name="th")
        nc.scalar.mul(th[:], t1[:], -0.5)
        return th, d

    def prep_ar(n, th, r0=0, rn=TAU):
        ar = prep.tile([TP, rn * F], bf16, tag="ar", name="ar")
        nc.vector.tensor_add(ar[:], th[:], ms[n][:, r0 * F:(r0 + rn) * F])
        return ar

    def products(n, ar, d, r0, rn):
        """4 plain-mul product tiles for tau rows [r0, r0+rn) (tile-local)."""
        i, j = divmod(n, 3)

        def xv(nm):
            return planes[nm].rearrange("p (s w) -> p s w", w=SROW)[
                :, i + r0:i + r0 + rn, j:j + F]

        a8 = ar.rearrange("p (r w) -> p r w", w=F)
        d8 = d.rearrange("p (r w) -> p r w", w=F)
        w = rn * F
        p0 = prod.tile([TP, w], bf16, tag="P", name="p0")
        p1 = prod.tile([TP, w], bf16, tag="P", name="p1")
        p2 = prod.tile([TP, w], bf16, tag="P", name="p2")
        p3 = prod.tile([TP, w], bf16, tag="P", name="p3")
        nc.vector.tensor_mul(p0.rearrange("p (r w) -> p r w", w=F), a8, xv("xr"))
        nc.vector.tensor_mul(p1.rearrange("p (r w) -> p r w", w=F), a8, xv("xi"))
        nc.vector.tensor_mul(p2.rearrange("p (r w) -> p r w", w=F), d8, xv("xin"))
        nc.vector.tensor_mul(p3.rearrange("p (r w) -> p r w", w=F), d8, xv("xrs"))
        return p0, p1, p2, p3

    def accum_main(n, tiles, r0, rn, last):
        """PE-accumulate tau rows [r0, r0+rn) of the products into PSUM."""
        p0, p1, p2, p3 = tiles
        first = n == 0
        for c in range(r0 // 2, (r0 + rn) // 2):
            lo = c * 2 - r0
            for acc, pa, pb in ((accr_c[c], p0, p2), (acci_c[c], p1, p3)):
                for k, pt in enumerate((pa, pb)):
                    pv = pt.rearrange("p (r w) -> p r w", w=F)[
                        :, lo:lo + 2, 0:256]
                    nc.tensor.matmul(
                        acc[:], idw, pv,
                        start=(first and k == 0),
                        stop=(last and k == 1),
                    )

    def accum_sb(n, tiles, r0, rn):
        if n % 3 == 2:
            return
        p0, p1, p2, p3 = tiles
        for acc, pa, pb in ((sbr, p0, p2), (sbi, p1, p3)):
            for pt in (pa, pb):
                pv = pt.rearrange("p (r w) -> p r w", w=F)[:, 0:rn, 256]
                nc.vector.tensor_add(acc[:, r0:r0 + rn], acc[:, r0:r0 + rn], pv)

    # drained accumulators in (f-major, tau-minor) bf16 layout: adjacent tau
    # pairs (t=8p+2q, +1) are then adjacent bytes, so the output transposes
    # can run on fp32-reinterpreted PAIRS (psum matmul writes need 4B align).
    acc_s = [
        const.tile([TP, AW], bf16, tag="accr_s", name="accr_s"),
        const.tile([TP, AW], bf16, tag="acci_s", name="acci_s"),
    ]
    acc32 = [a.bitcast(f32) for a in acc_s]

    yo01 = yop.tile([128, 2 * T * 2], bf16, tag="yo01", name="yo01")
    yviews = [
        yo01[:, 0:T * 2].rearrange("f (t c) -> f t c", c=2),
        yo01[:, T * 2:].rearrange("f (t c) -> f t c", c=2),
    ]

    # output collector psum banks: one per (f-half, comp); the 4 fp32 pair
    # transposes land strided (pair position = 4p+q fp32) into one bank,
    # leaving it t-contiguous bf16; then ONE copy into yo01.
    # Banks are reused in drain-retirement order (quarter c frees c and 4+c).
    OBANK = {(0, 0): 0, (0, 1): 4, (1, 0): 1, (1, 1): 5}
    obank = {}

    def drain_chunk(c):
        for comp, acc in ((0, accr_c[c]), (1, acci_c[c])):
            src = acc.rearrange("p (r f) -> p f r", r=2)
            dst = acc_s[comp].rearrange("p (f r) -> p f r", r=TAU)[
                :, :, 2 * c:2 * c + 2]
            nc.scalar.copy(dst, src)

    out_done = {k: 0 for k in OBANK}

    def out_rows(q):
        """After drain_chunk(q): transpose every tau PAIR that is both
        drained (pair <= q) and whose collector bank is retired (f0's
        banks 0/4 retire at q>=0, f1's banks 1/5 at q>=1)."""
        for ci in (0, 1):
            if q < ci:
                continue
            f0, fw = FCS[ci]
            for comp in (0, 1):
                key = (ci, comp)
                if key not in obank:
                    obank[key] = psum.tile(
                        [128, T // 2], f32, tag=BTAGS[OBANK[key]],
                        name=f"ob{ci}{comp}", padded_shape=[128, 512],
                    )
                ob = obank[key]
                a32 = acc32[comp].rearrange("p (f r) -> p f r", r=TAU // 2)
                for rp in range(out_done[key], q + 1):
                    nc.tensor.matmul(
                        ob[0:fw, rp:rp + 4 * (TP - 1) + 1:4],
                        a32[:, f0:f0 + fw, rp],
                        ident[0:TP, 0:TP],
                        is_transpose=True,
                        start=(rp == 0), stop=(rp == 3),
                    )
                out_done[key] = q + 1

    for n in range(C // 3):
        if n == 0:
            th0, d0 = prep_td(0)
            ar0, dd = prep_ar(0, th0), d0
        if n < 7:
            nxt_td = prep_td(n + 1)
            tiles = products(n, ar0, dd, 0, TAU)
            accum_main(n, tiles, 0, TAU, last=False)
            accum_sb(n, tiles, 0, TAU)
            ar0, dd = prep_ar(n + 1, nxt_td[0]), nxt_td[1]
        elif n == 7:
            tiles = products(n, ar0, dd, 0, TAU)
            accum_main(n, tiles, 0, TAU, last=False)
            accum_sb(n, tiles, 0, TAU)
        else:
            # tap 8 in tau-quarters; ACT th for quarter q+1 is emitted
            # before drain(q) so the ACT queue never stalls the DVE preps.
            qstate = []
            for q in range(4):
                thq, dq = prep_td(8, 2 * q, 2)
                arq = prep_ar(8, thq, 2 * q, 2)
                tiles = products(8, arq, dq, 2 * q, 2)
                accum_main(8, tiles, 2 * q, 2, last=True)
                if q > 0:
                    drain_chunk(q - 1)
                    out_rows(q - 1)
            drain_chunk(3)
            out_rows(3)

    # ---- one contiguous-psum-source copy per (f-half, comp), then stores.
    # DVE takes the real comps, ACT the imag; casting SWDGE stores per half.
    for ci in (0, 1):
        for comp in (0, 1):
            dst = yviews[ci][0:128, :, comp]
            src = obank[(ci, comp)].bitcast(bf16)[0:128, 0:T]
            if comp == 0:
                nc.vector.tensor_copy(dst, src)
            else:
                nc.scalar.copy(dst, src)
        nc.gpsimd.dma_start(
            y_ap[128 * ci:128 * (ci + 1)].rearrange("f t c -> f (t c)"),
            yo01[:, T * 2 * ci:T * 2 * (ci + 1)],
        )
    # f=256 sideband: direct casting scatter stores from the sb tiles
    ysb = y_ap[256].rearrange("(p t) c -> p t c", p=TP)
    nc.gpsimd.dma_start(ysb[:, :, 0], sbr[:])
    nc.gpsimd.dma_start(ysb[:, :, 1], sbi[:])


def _build():
    if "nc" in _CACHE:
        return _CACHE["nc"]
    from contextlib import ExitStack
    from concourse import bacc, mybir
    import concourse.tile as tile

    f32 = mybir.dt.float32
    nc = bacc.Bacc("TRN2", target_bir_lowering=False, debug=False, num_devices=B)
    m_d = nc.dram_tensor("m", (C, T, F), f32, kind="ExternalInput")
    x_d = nc.dram_tensor("x", (F, T, 2), f32, kind="ExternalInput")
    id_d = nc.dram_tensor("ident", (128, 128), f32, kind="ExternalInput")
    y_d = nc.dram_tensor("y", (F, T, 2), f32, kind="ExternalOutput")

    with tile.TileContext(nc) as tc:
        with ExitStack() as ctx:
            _emit(ctx, tc, m_d.ap(), x_d.ap(), id_d.ap(), y_d.ap())
    nc.compile()
    _CACHE["nc"] = nc
    return nc


def _in_maps(m, x):
    ident = np.eye(128, dtype=np.float32)
    return [
        {"m": np.ascontiguousarray(m[b]), "x": np.ascontiguousarray(x[b]),
         "ident": ident}
        for b in range(B)
    ]


def kernel(m, x, v, _trace=False):
    from concourse import bass_utils

    m = np.asarray(m, dtype=np.float32)
    x = np.asarray(x, dtype=np.float32)
    nc = _build()
    res = bass_utils.run_bass_kernel_spmd(
        nc, _in_maps(m, x), core_ids=list(range(B)), trace=_trace
    )
    kernel.last_results = res
    y = np.stack(
        [np.asarray(res.results[b]["y"], dtype=np.float32) for b in range(B)],
        axis=0,
    )
    return y


# revision 20
# speedup vs baseline: 1.5948x; 1.0638x over previous
"""Trainium2 Bass kernel for nn_CCM: per-pixel complex 3x3 conv mask.

Math (per batch element b, sharded 1 batch element per NeuronCore):
  y[t,f] = sum_{n=0..8} A_n[t,f] * X[t+i(n)-2, f+j(n)-1]   (complex)
with A_n = m_n + w * m_{9+n} + conj(w) * m_{18+n}, w = -1/2 + i*sqrt(3)/2:
  Ar_n = m_n - 0.5*(m_{9+n} + m_{18+n})
  Ai_n = s * (m_{9+n} - m_{18+n}),  s = sqrt(3)/2
X = xr + i*xi, zero padded (causal in t: 2 top; symmetric in f: 1,1).

v4 design (trace-driven, from v1-v3):
- The kernel is jointly limited by the 29.8MB fp32 load stream (~84us at
  358GB/s) and the DVE. Everything else is arranged to keep both saturated:
- Per-tap products accumulate into PSUM fp32 via identity-weight matmuls on
  the PE (removes all accumulation adds from the DVE; better precision than
  bf16 accumulators).
- The +-s basis scale lives in two extra SCALED x planes (xrs=+s*xr,
  xin=-s*xi); th=-0.5*(m9+m18) runs on ACT with one-tap lookahead so the
  DVE never waits. DVE per tap: t1, d, ar + 4 plain 2x tensor_tensor muls.
- x stages via HWDGE (sync) as fp32 so the SWDGE m-stream starts ~4us
  earlier; x transposes run in fp32, plane copies cast to bf16.
- PSUM holds 4096 fp32/partition but the output needs 2*2056 so the f=256
  column accumulates in SBUF via tiny strided DVE adds (j==2 taps are zero
  there and skipped); it stores via a direct scatter SWDGE cast DMA.
- Output: per (f-half, comp) ONE psum bank collects all 8 tau-row
  transposes (start=False accumulate-into-disjoint-elements), then ONE
  contiguous-source copy into the bf16 staging tile; casting SWDGE stores.
- Tap 8's m tiles load in tau-quarters so its products/dr ains/transposes
  pipeline with the last DMA arrivals.
- PSUM banks are time-shared via same-tag tile reuse: head x-transposes ->
  accumulators -> tail output collectors.
"""

import sys
import numpy as np

sys.path.insert(0, "/opt/trn_rl_repo")

B = 8
C = 27
T = 1000
F = 257
TP = 125          # partitions
TAU = 8           # t = 8*p + tau
NS = 10           # slots in x planes: tau in [-2, 8)
SROW = 260        # x plane slot row width (elements)
MW = TAU * F      # 2056: m / prod tile width (flat, rows of 257)
AW = TAU * 256    # 2048: psum accumulator width (rows of 256)
PLW = NS * SROW   # 2600: x plane width
QW = 2 * F        # 514: one tau-quarter of a flat m plane
SQ3H = float(np.sqrt(3.0) / 2.0)

_CACHE = {}


def _emit(ctx, tc, m_ap, x_ap, id_ap, y_ap):
    import concourse.mybir as mybir

    nc = tc.nc
    f32 = mybir.dt.float32
    bf16 = mybir.dt.bfloat16
    FCS = [(0, 128), (128, 128), (256, 1)]   # f chunks for transposes
    SLOT_GROUPS = [(0, 4), (4, 4), (8, 2)]   # batches of slots per psum tile

    const = ctx.enter_context(tc.tile_pool(name="const", bufs=1))
    mcpool = ctx.enter_context(tc.tile_pool(name="mc", bufs=1))
    xpool = tc.alloc_tile_pool(name="xstage", bufs=1)
    # One PSUM pool; the 8 banks rotate roles via tag reuse:
    # head ptg transposes -> accr/acci accumulators -> tail out collectors.
    psum = ctx.enter_context(tc.tile_pool(name="psum", bufs=1, space="PSUM"))
    BTAGS = [f"bank{i}" for i in range(8)]

    # ---- identb via casting SWDGE first (tiny, unblocks PE transposes);
    # ident f32 via HWDGE sync (slow startup is fine: only the tail's fp32
    # pair-transposes read it). x staging via casting SWDGE ahead of m.
    identb = const.tile([128, 128], bf16, tag="identb")
    nc.gpsimd.dma_start(identb[:], id_ap)
    ident = const.tile([128, 128], f32, tag="ident")
    nc.sync.dma_start(ident[:], id_ap)

    xns = []
    for ci, (f0, fw) in enumerate(FCS):
        xn = xpool.tile([fw, (T + 2) * 2], bf16, tag=f"xn{f0}", name=f"xn{f0}")
        nc.vector.memset(xn[:, 0:4], 0.0)
        nc.gpsimd.dma_start(
            xn[:, 4:], x_ap[f0:f0 + fw].rearrange("f t c -> f (t c)")
        )
        xns.append(xn)

    # ---- m loads: casting SWDGE, one pair DMA + one single DMA per tap;
    # tap 8 loads in tau-quarters so the tail overlaps the last arrivals.
    mp = {}
    ms = {}

    def load_m(n):
        p = mcpool.tile([TP, 2 * MW], bf16, tag=f"mp{n}", name=f"mp{n}")
        s = mcpool.tile([TP, MW], bf16, tag=f"ms{n}", name=f"ms{n}")
        psrc = m_ap[9 + n:19 + n:9].rearrange("c (p t) f -> p c (t f)", p=TP)
        ssrc = m_ap[n].rearrange("(p t) f -> p (t f)", p=TP)
        pv = p.rearrange("p (c w) -> p c w", c=2)
        if n < 8:
            nc.gpsimd.dma_start(pv, psrc)
            nc.gpsimd.dma_start(s[:], ssrc)
        else:
            for q in range(4):
                sl = slice(QW * q, QW * (q + 1))
                nc.gpsimd.dma_start(pv[:, :, sl], psrc[:, :, sl])
                nc.gpsimd.dma_start(s[:, sl], ssrc[:, sl])
        mp[n], ms[n] = p, s

    for n in range(9):
        load_m(n)

    # ---- x planes (bf16): f origin at col 1, zero pads at cols 0, 258, 259.
    # xr/xi are plain; xrs = +s*xr and xin = -s*xi carry the basis scale.
    planes = {}
    for nm in ("xr", "xi", "xrs", "xin"):
        p = const.tile([TP, PLW], bf16, tag=nm, name=nm)
        if nm in ("xr", "xi"):
            pv = p.rearrange("p (s w) -> p s w", w=SROW)
            nc.vector.memset(pv[:, :, 0:1], 0.0)
            nc.vector.memset(pv[:, :, 258:260], 0.0)
        planes[nm] = p

    # sideband (f=256) accumulators, one per complex component
    sbr = const.tile([TP, TAU], bf16, tag="sbr")
    sbi = const.tile([TP, TAU], bf16, tag="sbi")
    nc.vector.memset(sbr[:], 0.0)
    nc.vector.memset(sbi[:], 0.0)

    # ---- transpose x into the planes (fp32 PE transposes into psum banks).
    # Copies cast fp32->bf16: slot groups g0 on the idle DVE, g1/g2 on ACT.
    # Scaled planes built per group right after (DVE for g0, ACT for rest).
    bank_rr = 0
    for gi, (g0, gn) in enumerate(SLOT_GROUPS):
        for ci, (f0, fw) in enumerate(FCS):
            xn3 = xns[ci].rearrange("f (t c) -> f t c", c=2)
            for q, nm in enumerate(("xr", "xi")):
                pA = planes[nm].rearrange("p (s w) -> p s w", w=SROW)
                ptg = psum.tile(
                    [TP, 512], bf16, tag=BTAGS[bank_rr % 8], name="ptg",
                    padded_shape=[128, 1024],
                )
                bank_rr += 1
                for u in range(gn):
                    ts = g0 + u
                    nc.tensor.transpose(
                        ptg[0:TP, 128 * u:128 * u + fw],
                        xn3[0:fw, ts:ts + TAU * (TP - 1) + 1:TAU, q],
                        identb[0:fw, 0:fw],
                    )
                src = ptg.rearrange("p (u w) -> p u w", w=128)[0:TP, 0:gn, 0:fw]
                dst = pA[:, g0:g0 + gn, 1 + f0:1 + f0 + fw]
                if gi == 0:
                    nc.vector.tensor_copy(dst, src)
                else:
                    nc.scalar.copy(dst, src)
        # scaled planes for this slot group
        for src_nm, dst_nm, sc in (("xr", "xrs", SQ3H), ("xi", "xin", -SQ3H)):
            sv = planes[src_nm].rearrange("p (s w) -> p s w", w=SROW)
            dv = planes[dst_nm].rearrange("p (s w) -> p s w", w=SROW)
            if gi == 0:
                nc.vector.tensor_scalar_mul(
                    dv[:, g0:g0 + gn, :], sv[:, g0:g0 + gn, :], sc)
            else:
                nc.scalar.mul(dv[:, g0:g0 + gn, :], sv[:, g0:g0 + gn, :], sc)
    xpool.release()

    prep = ctx.enter_context(tc.tile_pool(name="prep", bufs=2))
    prod = ctx.enter_context(tc.tile_pool(name="prod", bufs=4))
    yop = ctx.enter_context(tc.tile_pool(name="yop", bufs=1))

    # ---- PSUM accumulators: accr rows of 256 in banks 0-3, acci in 4-7.
    # 512 fp32 = one bank = two tau rows.
    accr_c = [
        psum.tile([TP, 512], f32, tag=BTAGS[c], name=f"accr{c}",
                  padded_shape=[128, 512])
        for c in range(4)
    ]
    acci_c = [
        psum.tile([TP, 512], f32, tag=BTAGS[4 + c], name=f"acci{c}",
                  padded_shape=[128, 512])
        for c in range(4)
    ]

    idw = identb[0:TP, 0:TP]

    def prep_td(n, r0=0, rn=TAU, th_on_dve=False):
        """DVE half of prep: t1 = m9+m18, d = m9-m18 (tile-local rows).
        th = -0.5*t1 runs on ACT (one-tap lookahead) except when the ACT
        queue is backlogged (first tap, during the head copies)."""
        sl = slice(r0 * F, (r0 + rn) * F)
        m9 = mp[n][:, 0:MW][:, sl]
        m18 = mp[n][:, MW:2 * MW][:, sl]
        t1 = prep.tile([TP, rn * F], bf16, tag="t1", name="t1")
        nc.vector.tensor_add(t1[:], m9, m18)
        d = prep.tile([TP, rn * F], bf16, tag="d", name="d")
        nc.vector.tensor_sub(d[:], m9, m18)
        th = prep.tile([TP, rn * F], bf16, tag="th", name="th")
        if th_on_dve:
            nc.vector.tensor_scalar_mul(th[:], t1[:], -0.5)
        else:
            nc.scalar.mul(th[:], t1[:], -0.5)
        return th, d

    def prep_ar(n, th, r0=0, rn=TAU):
        ar = prep.tile([TP, rn * F], bf16, tag="ar", name="ar")
        nc.vector.tensor_add(ar[:], th[:], ms[n][:, r0 * F:(r0 + rn) * F])
        return ar

    def products(n, ar, d, r0, rn):
        """4 plain-mul product tiles for tau rows [r0, r0+rn) (tile-local)."""
        i, j = divmod(n, 3)

        def xv(nm):
            return planes[nm].rearrange("p (s w) -> p s w", w=SROW)[
                :, i + r0:i + r0 + rn, j:j + F]

        a8 = ar.rearrange("p (r w) -> p r w", w=F)
        d8 = d.rearrange("p (r w) -> p r w", w=F)
        w = rn * F
        p0 = prod.tile([TP, w], bf16, tag="P", name="p0")
        p1 = prod.tile([TP, w], bf16, tag="P", name="p1")
        p2 = prod.tile([TP, w], bf16, tag="P", name="p2")
        p3 = prod.tile([TP, w], bf16, tag="P", name="p3")
        nc.vector.tensor_mul(p0.rearrange("p (r w) -> p r w", w=F), a8, xv("xr"))
        nc.vector.tensor_mul(p1.rearrange("p (r w) -> p r w", w=F), a8, xv("xi"))
        nc.vector.tensor_mul(p2.rearrange("p (r w) -> p r w", w=F), d8, xv("xin"))
        nc.vector.tensor_mul(p3.rearrange("p (r w) -> p r w", w=F), d8, xv("xrs"))
        return p0, p1, p2, p3

    def accum_main(n, tiles, r0, rn, last):
        """PE-accumulate tau rows [r0, r0+rn) of the products into PSUM."""
        p0, p1, p2, p3 = tiles
        first = n == 0
        for c in range(r0 // 2, (r0 + rn) // 2):
            lo = c * 2 - r0
            for acc, pa, pb in ((accr_c[c], p0, p2), (acci_c[c], p1, p3)):
                for k, pt in enumerate((pa, pb)):
                    pv = pt.rearrange("p (r w) -> p r w", w=F)[
                        :, lo:lo + 2, 0:256]
                    nc.tensor.matmul(
                        acc[:], idw, pv,
                        start=(first and k == 0),
                        stop=(last and k == 1),
                    )

    def accum_sb(n, tiles, r0, rn):
        if n % 3 == 2:
            return
        p0, p1, p2, p3 = tiles
        for acc, pa, pb in ((sbr, p0, p2), (sbi, p1, p3)):
            for pt in (pa, pb):
                pv = pt.rearrange("p (r w) -> p r w", w=F)[:, 0:rn, 256]
                nc.vector.tensor_add(acc[:, r0:r0 + rn], acc[:, r0:r0 + rn], pv)

    # drained accumulators in (f-major, tau-minor) bf16 layout: adjacent tau
    # pairs (t=8p+2q, +1) are then adjacent bytes, so the output transposes
    # can run on fp32-reinterpreted PAIRS (psum matmul writes need 4B align).
    acc_s = [
        const.tile([TP, AW], bf16, tag="accr_s", name="accr_s"),
        const.tile([TP, AW], bf16, tag="acci_s", name="acci_s"),
    ]
    acc32 = [a.bitcast(f32) for a in acc_s]

    yo01 = yop.tile([128, 2 * T * 2], bf16, tag="yo01", name="yo01")
    yviews = [
        yo01[:, 0:T * 2].rearrange("f (t c) -> f t c", c=2),
        yo01[:, T * 2:].rearrange("f (t c) -> f t c", c=2),
    ]

    # output collector psum banks: one per (f-half, comp); the 4 fp32 pair
    # transposes land strided (pair position = 4p+q fp32) into one bank,
    # leaving it t-contiguous bf16; then ONE copy into yo01.
    # Banks are reused in drain-retirement order (quarter c frees c and 4+c).
    OBANK = {(0, 0): 0, (0, 1): 4, (1, 0): 1, (1, 1): 5}
    obank = {}

    def drain_chunk(c):
        for comp, acc in ((0, accr_c[c]), (1, acci_c[c])):
            src = acc.rearrange("p (r f) -> p f r", r=2)
            dst = acc_s[comp].rearrange("p (f r) -> p f r", r=TAU)[
                :, :, 2 * c:2 * c + 2]
            nc.scalar.copy(dst, src)

    out_done = {k: 0 for k in OBANK}

    def out_rows(q):
        """After drain_chunk(q): transpose every tau PAIR that is both
        drained (pair <= q) and whose collector bank is retired (f0's
        banks 0/4 retire at q>=0, f1's banks 1/5 at q>=1)."""
        for ci in (0, 1):
            if q < ci:
                continue
            f0, fw = FCS[ci]
            for comp in (0, 1):
                key = (ci, comp)
                if key not in obank:
                    obank[key] = psum.tile(
                        [128, T // 2], f32, tag=BTAGS[OBANK[key]],
                        name=f"ob{ci}{comp}", padded_shape=[128, 512],
                    )
                ob = obank[key]
                a32 = acc32[comp].rearrange("p (f r) -> p f r", r=TAU // 2)
                for rp in range(out_done[key], q + 1):
                    # each strided transpose is its own single-matmul group
                    # (start=False accumulation into untouched elements is
                    # not safe; disjoint start=True writes are).
                    nc.tensor.matmul(
                        ob[0:fw, rp:rp + 4 * (TP - 1) + 1:4],
                        a32[:, f0:f0 + fw, rp],
                        ident[0:TP, 0:TP],
                        is_transpose=True,
                    )
                out_done[key] = q + 1

    for n in range(C // 3):
        if n == 0:
            th0, d0 = prep_td(0, th_on_dve=True)
            ar0, dd = prep_ar(0, th0), d0
        if n < 7:
            nxt_td = prep_td(n + 1)
            tiles = products(n, ar0, dd, 0, TAU)
            accum_main(n, tiles, 0, TAU, last=False)
            accum_sb(n, tiles, 0, TAU)
            ar0, dd = prep_ar(n + 1, nxt_td[0]), nxt_td[1]
        elif n == 7:
            tiles = products(n, ar0, dd, 0, TAU)
            accum_main(n, tiles, 0, TAU, last=False)
            accum_sb(n, tiles, 0, TAU)
        else:
            # tap 8 in tau-quarters; ACT th for quarter q+1 is emitted
            # before drain(q) so the ACT queue never stalls the DVE preps.
            qstate = []
            for q in range(4):
                thq, dq = prep_td(8, 2 * q, 2)
                arq = prep_ar(8, thq, 2 * q, 2)
                tiles = products(8, arq, dq, 2 * q, 2)
                accum_main(8, tiles, 2 * q, 2, last=True)
                if q > 0:
                    drain_chunk(q - 1)
                    out_rows(q - 1)
            drain_chunk(3)
            out_rows(3)

    # ---- one contiguous-psum-source copy per (f-half, comp), then stores.
    # DVE takes the real comps, ACT the imag; casting SWDGE stores per half.
    for ci in (0, 1):
        for comp in (0, 1):
            dst = yviews[ci][0:128, :, comp]
            src = obank[(ci, comp)].bitcast(bf16)[0:128, 0:T]
            if comp == 0:
                nc.vector.tensor_copy(dst, src)
            else:
                nc.scalar.copy(dst, src)
        nc.gpsimd.dma_start(
            y_ap[128 * ci:128 * (ci + 1)].rearrange("f t c -> f (t c)"),
            yo01[:, T * 2 * ci:T * 2 * (ci + 1)],
        )
    # f=256 sideband: direct casting scatter stores from the sb tiles
    ysb = y_ap[256].rearrange("(p t) c -> p t c", p=TP)
    nc.gpsimd.dma_start(ysb[:, :, 0], sbr[:])
    nc.gpsimd.dma_start(ysb[:, :, 1], sbi[:])


def _build():
    if "nc" in _CACHE:
        return _CACHE["nc"]
    from contextlib import ExitStack
    from concourse import bacc, mybir
    import concourse.tile as tile

    f32 = mybir.dt.float32
    nc = bacc.Bacc("TRN2", target_bir_lowering=False, debug=False, num_devices=B)
    m_d = nc.dram_tensor("m", (C, T, F), f32, kind="ExternalInput")
    x_d = nc.dram_tensor("x", (F, T, 2), f32, kind="ExternalInput")
    id_d = nc.dram_tensor("ident", (128, 128), f32, kind="ExternalInput")
    y_d = nc.dram_tensor("y", (F, T, 2), f32, kind="ExternalOutput")

    with tile.TileContext(nc) as tc:
        with ExitStack() as ctx:
            _emit(ctx, tc, m_d.ap(), x_d.ap(), id_d.ap(), y_d.ap())
    nc.compile()
    _CACHE["nc"] = nc
    return nc


def _in_maps(m, x):
    ident = np.eye(128, dtype=np.float32)
    return [
        {"m": np.ascontiguousarray(m[b]), "x": np.ascontiguousarray(x[b]),
         "ident": ident}
        for b in range(B)
    ]


def kernel(m, x, v, _trace=False):
    from concourse import bass_utils

    m = np.asarray(m, dtype=np.float32)
    x = np.asarray(x, dtype=np.float32)
    nc = _build()
    res = bass_utils.run_bass_kernel_spmd(
        nc, _in_maps(m, x), core_ids=list(range(B)), trace=_trace
    )
    kernel.last_results = res
    y = np.stack(
        [np.asarray(res.results[b]["y"], dtype=np.float32) for b in range(B)],
        axis=0,
    )
    return y
